# revision 23
# baseline (speedup 1.0000x reference)
"""AF2GNN1 distributed Trainium2 kernel.

Sharding: pixels (65536) row-sharded 8192/core for the two big Q matmuls
(phase A: sprawT = xq.T @ Q partials; phase C: outT = y.T @ Q.T). One
AllReduce replicates the summed superpixel features on every core; the
GCN branch then runs fully replicated (full adj.T resident in SBUF, no
collectives), while the GAT attention e-ops stay node-sharded (256
nodes/core, transposed layout) using a per-core one-hot selector for
the f1 logit row. Two small AllGathers remain: output-attention Wh2 and
the final y. Phase C streams host-pretransposed Q.T tiles (bf16).
"""
import sys

sys.path.insert(0, "/opt/trn_rl_repo")

import numpy as np
import ml_dtypes
from contextlib import ExitStack

import concourse.bass as bass
import concourse.mybir as mybir
import concourse.tile as tile
from concourse import bacc
from concourse.bass_utils import run_bass_kernel_spmd
from concourse.masks import make_identity

F32 = mybir.dt.float32
BF16 = mybir.dt.bfloat16
AF = mybir.ActivationFunctionType
ALU = mybir.AluOpType

NCORES = 8
HW = 65536
C = 200          # channels (B in reference)
N = 2048         # superpixels / nodes
NHID = 128
OUT = 64
HEADS = 4
P = HW // NCORES          # 8192 pixels per core
PT = P // 128             # 64 pixel tiles
NS = N // NCORES          # 256 nodes per core
JT = N // 128             # 16 node tiles
BN = 1.0 / np.sqrt(1.0 + 1e-5)
CP = 225          # x channels (200) + zero pad (24) + ones col at 224
C1W = CP - 128    # phase-A chunk-1 width (97; colsum at its row 96)
C1 = C - 128      # feature chunk-1 width (72)
QTB = 6           # phase-C Q.T tile ring depth

_CACHE = {}


def _build():
    nc = bacc.Bacc("TRN2", target_bir_lowering=False, debug=False,
                   num_devices=NCORES)

    def din(name, shape, dt):
        return nc.dram_tensor(name, list(shape), dt, kind="ExternalInput")

    xq_d = din("xq", [P, CP], BF16)        # x shard + zero pad + ones column
    q_d = din("q", [P, N], BF16)               # Q pixel shard
    qt_d = din("qt", [N, P], BF16)             # Q pixel shard, pre-transposed
    adjs_d = din("adjs", [N, NS], BF16)        # adj[shard].T (mask / final gcn)
    adjf_d = din("adjf", [N, N], BF16)         # full adj.T (replicated GCN)
    sel_d = din("sel", [N, NS], BF16)          # one-hot node-shard selector
    w1a_d = din("w1a", [C, NHID], BF16)        # BN^3 * g1a_W
    w1b_d = din("w1b", [NHID, C], BF16)        # BN * g1b_W
    w2a_d = din("w2a", [C, NHID], BF16)        # BN * g2a_W
    w2b_d = din("w2b", [NHID, OUT], BF16)      # BN * g2b_W
    gatw_d = din("gatw", [C, HEADS * NHID], BF16)   # BN * gat_W, heads on cols
    wa12_d = din("wa12", [C, 2 * HEADS], BF16)      # BN * Wk@a1 | BN * Wk@a2
    outw_d = din("outw", [HEADS * NHID, OUT], BF16)
    outa1_d = din("outa1", [OUT, 128], BF16)   # out_a[:64] replicated
    outa2_d = din("outa2", [OUT, 1], BF16)     # out_a[64:]
    b1a_d = din("b1a", [NHID, 1], F32)
    b1b_d = din("b1b", [C, 1], F32)            # BN * g1b_b
    b2a_d = din("b2a", [NHID, 1], F32)
    b2b_d = din("b2b", [OUT, 1], F32)          # BN * g2b_b
    fuse_d = din("fuse", [OUT, 6], F32)  # wfa, wfb, cbv, cw0, cw1, cwsum
    out_d = nc.dram_tensor("out", [OUT, P], F32, kind="ExternalOutput")

    RG = [list(range(NCORES))]

    with tile.TileContext(nc) as tc, ExitStack() as ctx:
        _sc = [None]

        def mark(name):
            if _sc[0] is not None:
                _sc[0].__exit__(None, None, None)
                _sc[0] = None
            if name:
                _sc[0] = tc.spectator_scope(name)
                _sc[0].__enter__()

        dram = ctx.enter_context(tc.tile_pool(name="dram", bufs=1, space="DRAM"))

        # ---- constants / weights ----
        cons = ctx.enter_context(tc.tile_pool(name="cons", bufs=1))
        idf = cons.tile([128, 128], F32)
        make_identity(nc, idf[:, :])
        idb = cons.tile([128, 128], BF16)
        make_identity(nc, idb[:, :])
        ones_row_f = cons.tile([1, 128], F32)
        nc.vector.memset(ones_row_f[:, :], 1.0)
        ones_col = cons.tile([128, 1], BF16)
        nc.vector.memset(ones_col[:, :], 1.0)
        negbn = cons.tile([64, 1], F32)
        nc.vector.memset(negbn[:, :], -BN)

        _ld = [0]
        def load(dst_shape, src, dt=BF16, eng=None):
            _ld[0] += 1
            t = cons.tile(dst_shape, dt, name=f"w{_ld[0]}")
            (eng or nc.gpsimd).dma_start(t[:], src)
            return t

        w1a = (load([128, NHID], w1a_d[0:128, :]),
               load([C1, NHID], w1a_d[128:C, :]))
        w1b = load([NHID, C], w1b_d[:, :])
        w2a = (load([128, NHID], w2a_d[0:128, :]),
               load([C1, NHID], w2a_d[128:C, :]))
        w2b = load([NHID, OUT], w2b_d[:, :])
        gatw = (load([128, HEADS * NHID], gatw_d[0:128, :]),
                load([C1, HEADS * NHID], gatw_d[128:C, :]))
        wa12 = (load([128, 2 * HEADS], wa12_d[0:128, :]),
                load([C1, 2 * HEADS], wa12_d[128:C, :]))
        outw = [load([128, OUT], outw_d[c4 * 128:(c4 + 1) * 128, :])
                for c4 in range(HEADS)]
        outa1 = load([OUT, 128], outa1_d[:, :])
        outa2 = load([OUT, 1], outa2_d[:, :])
        b1a = load([NHID, 1], b1a_d[:, :], F32)
        b1b = (load([128, 1], b1b_d[0:128, :], F32),
               load([C1, 1], b1b_d[128:C, :], F32))
        b2a = load([NHID, 1], b2a_d[:, :], F32)
        b2b = load([OUT, 1], b2b_d[:, :], F32)
        fuse = load([OUT, 6], fuse_d[:, :], F32)
        adjs = cons.tile([128, JT, NS], BF16)
        nc.gpsimd.dma_start(adjs[:, :, :],
                            adjs_d[:, :].rearrange("(jt p) i -> p jt i", p=128))
        selT = cons.tile([128, JT, NS], BF16)
        nc.gpsimd.dma_start(selT[:, :, :],
                            sel_d[:, :].rearrange("(jt p) i -> p jt i", p=128))
        adjf = cons.tile([128, JT, N], BF16)
        nc.gpsimd.dma_start(adjf[:, :, :],
                            adjf_d[:, :].rearrange("(jt p) i -> p jt i", p=128))

        sbC = ctx.enter_context(tc.tile_pool(name="sbC", bufs=1))
        qtts = [[None] * 4 for _ in range(JT)]
        mark("A")
        # ---- phase A: sprawT[c, n] = sum_p xq[p, c] * Q[p, n] in PSUM ----
        rsIn = dram.tile([CP, N], F32)
        with tc.tile_pool(name="sbr", bufs=1) as sbr:
          with tc.tile_pool(name="psA", bufs=1, space="PSUM") as psA, \
               tc.tile_pool(name="sbA", bufs=5) as sbA:
            psA0 = psA.tile([128, N], F32, tag="a0")
            psA1 = psA.tile([128, N], F32, tag="a1")
            NI = PT // 2
            for i in range(NI):
                qt = sbA.tile([128, 2, N], BF16, tag="qt")
                nc.scalar.dma_start(
                    qt[:, :, :],
                    q_d[i * 256:(i + 1) * 256, :].rearrange(
                        "(t p) n -> p t n", p=128))
                xt = sbA.tile([128, 2, CP], BF16, tag="xt")
                nc.gpsimd.dma_start(
                    xt[:, :, :],
                    xq_d[i * 256:(i + 1) * 256, :].rearrange(
                        "(t p) n -> p t n", p=128))
                for t2 in range(2):
                    st, sp = (i == 0 and t2 == 0), (i == NI - 1 and t2 == 1)
                    for nb in range(4):
                        nsl = slice(nb * 512, (nb + 1) * 512)
                        nc.tensor.matmul(psA0[:, nsl], xt[:, t2, 0:128],
                                         qt[:, t2, nsl], start=st, stop=sp)
                    for nb in range(4):
                        nsl = slice(nb * 512, (nb + 1) * 512)
                        nc.tensor.matmul(psA1[0:C1W, nsl],
                                         xt[:, t2, 128:CP], qt[:, t2, nsl],
                                         start=st, stop=sp)

            spr0 = sbr.tile([128, N], F32)
            nc.vector.tensor_copy(spr0[:, :], psA0[:, :])
            spr1 = sbr.tile([C1W, N], F32)
            nc.vector.tensor_copy(spr1[:, :], psA1[0:C1W, :])

            # prefetch Q.T tiles for phase C (ring drains during GNN + C)
            QP = P // 4
            for qr in range(4):
                for jt in range(JT):
                    qq = sbC.tile([128, QP], BF16, tag="qtt", bufs=QTB,
                                  name=f"qtt{jt}_{qr}")
                    nc.sync.dma_start(
                        qq[:, :],
                        qt_d[jt * 128:(jt + 1) * 128,
                             qr * QP:(qr + 1) * QP])
                    qtts[jt][qr] = qq

          mark("a2a")
          nc.gpsimd.dma_start(rsIn[0:128, :], spr0[:, :])
          nc.gpsimd.dma_start(rsIn[128:CP, :], spr1[:, :])

        arOut = dram.tile([CP, N], F32, addr_space="Shared")
        nc.gpsimd.collective_compute(
            "AllReduce", ALU.add, replica_groups=RG,
            ins=[rsIn.opt()], outs=[arOut.opt()])

        # ---- replicated sp prep: normalize full-width, T layout ----
        gnn = ctx.enter_context(tc.tile_pool(name="gnn", bufs=1))
        spT_b = [gnn.tile([128, N], BF16, name="spTb0"),
                 gnn.tile([C1, N], BF16, name="spTb1")]
        ysb = gnn.tile([128, JT, OUT], BF16)

        with tc.tile_pool(name="psP", bufs=1, space="PSUM") as psP, \
             tc.tile_pool(name="sbP", bufs=1) as sbP:
            sum0 = sbP.tile([128, N], F32, name="sum0")
            nc.gpsimd.dma_start(sum0[:, :], arOut[0:128, :])
            sum1 = sbP.tile([C1W, N], F32, name="sum1")
            nc.gpsimd.dma_start(sum1[:, :], arOut[128:CP, :])
            rec_row = sbP.tile([1, N], F32, name="recrow")
            nc.vector.reciprocal(rec_row[:, :], sum1[96:97, :])
            recb_ps = psP.tile([128, N], F32, name="recbps")
            for fb in range(4):
                fs = slice(fb * 512, (fb + 1) * 512)
                nc.tensor.matmul(recb_ps[:, fs], ones_row_f[:, :],
                                 rec_row[:, fs], start=True, stop=True)
            nc.vector.tensor_mul(spT_b[0][:, :], sum0[:, :], recb_ps[:, :])
            nc.vector.tensor_mul(spT_b[1][:, :], sum1[0:C1, :],
                                 recb_ps[0:C1, :])

        gnn_blk = ExitStack()
        psG = gnn_blk.enter_context(tc.tile_pool(name="psG", bufs=1,
                                                 space="PSUM"))
        PSB = {"big": 1, "tp": 2, "att": 1, "sm": 1}
        _pn = [0]
        def pstile(shape, dt, tag):
            _pn[0] += 1
            return psG.tile(shape, dt, tag=tag, bufs=PSB[tag],
                            name=f"ps_{tag}_{_pn[0]}")
        sbG = gnn_blk.enter_context(tc.tile_pool(name="sbG", bufs=1))

        mark("gat")
        # ---- GAT precompute: f12 = wa12.T @ spT (f1 rows 0:4, f2 4:8) ----
        f12_ps = pstile([8, N], F32, "big")
        for ci in range(2):
            for fb in range(4):
                fs = slice(fb * 512, (fb + 1) * 512)
                nc.tensor.matmul(f12_ps[:, fs], wa12[ci][:, :],
                                 spT_b[ci][:, fs],
                                 start=(ci == 0), stop=(ci == 1))
        f12sb = sbG.tile([8, N], BF16, tag="f12")
        nc.vector.tensor_copy(f12sb[:, :], f12_ps[:, :])
        f12T = gnn.tile([128, JT, 2 * HEADS], F32)    # bias source (f2)
        f12Tb = gnn.tile([128, JT, 2 * HEADS], BF16)  # matmul operand
        for jt in range(JT):
            jsl = slice(jt * 128, (jt + 1) * 128)
            pt8 = pstile([128, 8], BF16, "tp")
            nc.tensor.transpose(pt8[0:128, 0:8], f12sb[:, jsl], idb[0:8, 0:8])
            nc.vector.tensor_copy(f12T[:, jt, :], pt8[0:128, 0:8])
            nc.vector.tensor_copy(f12Tb[:, jt, :], pt8[0:128, 0:8])
        # my-shard f1 row via one-hot selector, then broadcast to 128 rows
        f1b = []
        for k in range(HEADS):
            f1r_ps = pstile([1, NS], F32, "sm")
            for jt in range(JT):
                nc.tensor.matmul(f1r_ps[:, :], f12Tb[:, jt, k:k + 1],
                                 selT[:, jt, :],
                                 start=(jt == 0), stop=(jt == JT - 1))
            f1r = sbG.tile([1, NS], F32, tag="f1my", name=f"f1r{k}")
            nc.vector.tensor_copy(f1r[:, :], f1r_ps[:, :])
            fb_ps = pstile([128, NS], F32, "tp")
            nc.tensor.matmul(fb_ps[:, :], ones_row_f[:, :], f1r[:, :],
                             start=True, stop=True)
            t = gnn.tile([128, NS], BF16, name=f"f1b{k}")
            nc.vector.tensor_copy(t[:, :], fb_ps[:, :])
            f1b.append(t)
        # Wh in natural layout per head
        whn = []
        for k in range(HEADS):
            wt = gnn.tile([128, JT, NHID], BF16, name=f"whn{k}")
            ksl = slice(k * NHID, (k + 1) * NHID)
            for jt in range(JT):
                jsl = slice(jt * 128, (jt + 1) * 128)
                wp = pstile([128, NHID], F32, "tp")
                nc.tensor.matmul(wp[:, :], spT_b[0][:, jsl], gatw[0][:, ksl],
                                 start=True, stop=False)
                nc.tensor.matmul(wp[:, :], spT_b[1][:, jsl], gatw[1][:, ksl],
                                 start=False, stop=True)
                nc.vector.tensor_copy(wt[:, jt, :], wp[:, :])
            whn.append(wt)

        hcat = []

        def emit_head(k):
            uz = pstile([128, 2 * NS], F32, "att")  # unp | zp row
            for jb in range(2):
                jts = range(jb * 8, jb * 8 + 8)
                zs, ls, ems = {}, {}, {}
                for jt in jts:
                    zs[jt] = sbG.tile([128, NS], F32, tag="zsb", bufs=5,
                                      name=f"z{k}_{jt}")
                    nc.scalar.activation(zs[jt][:, :], f1b[k][:, :],
                                         AF.Identity,
                                         bias=f12T[:, jt, 4 + k:5 + k],
                                         scale=1.0)
                for jt in jts:
                    ls[jt] = sbG.tile([128, NS], F32, tag="lsb", bufs=5,
                                      name=f"l{k}_{jt}")
                    nc.vector.scalar_tensor_tensor(
                        ls[jt][:, :], zs[jt][:, :], 0.2, zs[jt][:, :],
                        op0=ALU.mult, op1=ALU.max)
                for jt in jts:
                    zs[jt] = sbG.tile([128, NS], BF16, tag="esb", bufs=4,
                                      name=f"e{k}_{jt}")
                    nc.scalar.activation(zs[jt][:, :], ls[jt][:, :], AF.Exp)
                for jt in jts:
                    ems[jt] = sbG.tile([128, NS], BF16, tag="em", bufs=4,
                                       name=f"m{k}_{jt}")
                    nc.vector.tensor_mul(ems[jt][:, :], zs[jt][:, :],
                                         adjs[:, jt, :])
                for jt in jts:
                    nc.tensor.matmul(uz[:, 0:NS], whn[k][:, jt, :],
                                     ems[jt][:, :], start=(jt == 0),
                                     stop=(jt == JT - 1))
                    nc.tensor.matmul(uz[0:1, NS:2 * NS], ones_col[:, :],
                                     ems[jt][:, :], start=(jt == 0),
                                     stop=(jt == JT - 1))
            ziv = sbG.tile([1, NS], F32, tag="et", bufs=5, name=f"ziv{k}")
            nc.vector.reciprocal(ziv[:, :], uz[0:1, NS:2 * NS])
            zbc = pstile([128, NS], F32, "tp")
            nc.tensor.matmul(zbc[:, :], ones_row_f[:, :], ziv[:, :],
                             start=True, stop=True)
            zbs = sbG.tile([128, NS], F32, tag="et", bufs=5, name=f"zbs{k}")
            nc.vector.tensor_copy(zbs[:, :], zbc[:, :])
            ho = sbG.tile([128, NS], F32, tag="et", bufs=5, name=f"ho{k}")
            nc.vector.tensor_mul(ho[:, :], uz[:, 0:NS], zbs[:, :])
            # elu(ho) + 1 = max(ho,0) + exp(min(ho,0))
            mn0 = sbG.tile([128, NS], F32, tag="et", bufs=5, name=f"mn0{k}")
            nc.vector.tensor_scalar_min(mn0[:, :], ho[:, :], 0.0)
            ex = sbG.tile([128, NS], F32, tag="et", bufs=5, name=f"ex{k}")
            nc.scalar.activation(ex[:, :], mn0[:, :], AF.Exp)
            ep = sbG.tile([128, NS], F32, tag="et", bufs=5, name=f"ep{k}")
            nc.vector.scalar_tensor_tensor(ep[:, :], ho[:, :], 0.0, ex[:, :],
                                           op0=ALU.max, op1=ALU.add)
            hc = gnn.tile([128, NS], BF16, name=f"hc{k}")
            nc.vector.tensor_scalar_add(hc[:, :], ep[:, :], -1.0)
            hcat.append(hc)

        mark("gcn")
        # ================= replicated GCN =================
        # t1n[j, h] per node-chunk (natural)
        t1n = gnn.tile([128, JT, NHID], BF16)
        for jt in range(JT):
            jsl = slice(jt * 128, (jt + 1) * 128)
            tp1 = pstile([128, NHID], F32, "tp")
            nc.tensor.matmul(tp1[:, :], spT_b[0][:, jsl], w1a[0][:, :],
                             start=True, stop=False)
            nc.tensor.matmul(tp1[:, :], spT_b[1][:, jsl], w1a[1][:, :],
                             start=False, stop=True)
            nc.vector.tensor_copy(t1n[:, jt, :], tp1[:, :])
        # u1T = (adj @ t1).T  full width
        u1T = pstile([128, N], F32, "big")
        for jt in range(JT):
            for fb in range(4):
                fs = slice(fb * 512, (fb + 1) * 512)
                nc.tensor.matmul(u1T[:, fs], t1n[:, jt, :], adjf[:, jt, fs],
                                 start=(jt == 0), stop=(jt == JT - 1))
        emit_head(0)
        z2b = sbG.tile([128, N], BF16, tag="zTb")
        nc.scalar.activation(z2b[:, :], u1T[:, :], AF.Lrelu,
                             bias=b1a[:, :], scale=1.0, alpha=0.01)
        # t2n[j, c] natural
        t2n = gnn.tile([128, JT, C], BF16)
        for jt in range(JT):
            jsl = slice(jt * 128, (jt + 1) * 128)
            tp2 = pstile([128, C], F32, "tp")
            nc.tensor.matmul(tp2[:, :], z2b[:, jsl], w1b[:, :],
                             start=True, stop=True)
            nc.vector.tensor_copy(t2n[:, jt, :], tp2[:, :])
        # r1T (200 rows -> two full-width passes), xg2 = BN^2 sp + lrelu(...)
        xg2b = [sbG.tile([128, N], BF16, tag="xg0", name="xg2b0"),
                sbG.tile([C1, N], BF16, tag="xg1", name="xg2b1")]
        r1a = pstile([128, N], F32, "big")
        for jt in range(JT):
            for fb in range(4):
                fs = slice(fb * 512, (fb + 1) * 512)
                nc.tensor.matmul(r1a[:, fs], t2n[:, jt, 0:128],
                                 adjf[:, jt, fs],
                                 start=(jt == 0), stop=(jt == JT - 1))
        emit_head(1)
        y1t = sbG.tile([128, N], BF16, tag="y1t")
        nc.scalar.activation(y1t[:, :], r1a[:, :], AF.Lrelu,
                             bias=b1b[0][:, :], scale=BN, alpha=0.01)
        nc.vector.scalar_tensor_tensor(xg2b[0][:, :], spT_b[0][:, :],
                                       BN * BN, y1t[:, :],
                                       op0=ALU.mult, op1=ALU.add)
        r1b = pstile([128, N], F32, "big")
        for jt in range(JT):
            for fb in range(4):
                fs = slice(fb * 512, (fb + 1) * 512)
                nc.tensor.matmul(r1b[0:C1, fs], t2n[:, jt, 128:C],
                                 adjf[:, jt, fs],
                                 start=(jt == 0), stop=(jt == JT - 1))
        y1u = sbG.tile([C1, N], BF16, tag="y1t", name="y1u")
        nc.scalar.activation(y1u[:, :], r1b[0:C1, :], AF.Lrelu,
                             bias=b1b[1][:, :], scale=BN, alpha=0.01)
        nc.vector.scalar_tensor_tensor(xg2b[1][:, :], spT_b[1][:, :],
                                       BN * BN, y1u[:, :],
                                       op0=ALU.mult, op1=ALU.add)
        # t3n
        t3n = gnn.tile([128, JT, NHID], BF16)
        for jt in range(JT):
            jsl = slice(jt * 128, (jt + 1) * 128)
            tp3 = pstile([128, NHID], F32, "tp")
            nc.tensor.matmul(tp3[:, :], xg2b[0][:, jsl], w2a[0][:, :],
                             start=True, stop=False)
            nc.tensor.matmul(tp3[:, :], xg2b[1][:, jsl], w2a[1][:, :],
                             start=False, stop=True)
            nc.vector.tensor_copy(t3n[:, jt, :], tp3[:, :])
        u3T = pstile([128, N], F32, "big")
        for jt in range(JT):
            for fb in range(4):
                fs = slice(fb * 512, (fb + 1) * 512)
                nc.tensor.matmul(u3T[:, fs], t3n[:, jt, :], adjf[:, jt, fs],
                                 start=(jt == 0), stop=(jt == JT - 1))
        emit_head(2)
        z4b = sbG.tile([128, N], BF16, tag="zTb", name="z4b")
        nc.scalar.activation(z4b[:, :], u3T[:, :], AF.Lrelu,
                             bias=b2a[:, :], scale=1.0, alpha=0.01)
        t4n = gnn.tile([128, JT, OUT], BF16)
        for jt in range(JT):
            jsl = slice(jt * 128, (jt + 1) * 128)
            tp4 = pstile([128, OUT], F32, "tp")
            nc.tensor.matmul(tp4[:, :], z4b[:, jsl], w2b[:, :],
                             start=True, stop=True)
            nc.vector.tensor_copy(t4n[:, jt, :], tp4[:, :])
        # final hop sharded to my nodes via adjs
        gp = pstile([64, NS], F32, "sm")
        for jt in range(JT):
            nc.tensor.matmul(gp[:, :], t4n[:, jt, :], adjs[:, jt, :],
                             start=(jt == 0), stop=(jt == JT - 1))
        emit_head(3)
        gcnx = gnn.tile([64, NS], F32)
        nc.scalar.activation(gcnx[:, :], gp[:, :], AF.Lrelu,
                             bias=b2b[:, :], scale=BN, alpha=0.01)

        mark("outatt")
        # ---- output attention ----
        def transpose_to(dst, src_ap, pw):
            pt = pstile([128, 128], src_ap.dtype, "tp")
            nc.tensor.transpose(pt[0:128, 0:pw], src_ap, idb[0:pw, 0:pw])
            nc.vector.tensor_copy(dst, pt[0:128, 0:pw])

        wh2p = pstile([64, NS], F32, "att")
        for c4 in range(HEADS):
            nc.tensor.matmul(wh2p[:, :], outw[c4][:, :],
                             hcat[c4][:, :], start=(c4 == 0),
                             stop=(c4 == HEADS - 1))
        wh2b = gnn.tile([64, NS], BF16)
        nc.vector.tensor_copy(wh2b[:, :], wh2p[:, :])
        f1o_ps = pstile([128, NS], F32, "tp")
        nc.tensor.matmul(f1o_ps[:, :], outa1[:, :], wh2b[:, :],
                         start=True, stop=True)
        f1ob = sbG.tile([128, NS], F32, tag="f1ob")
        nc.vector.tensor_copy(f1ob[:, :], f1o_ps[:, :])
        agI2 = gnn.tile([128, 2, OUT + 1], BF16)
        for it in range(2):
            isl = slice(it * 128, (it + 1) * 128)
            f2p = pstile([128, 1], F32, "sm")
            nc.tensor.matmul(f2p[:, :], wh2b[:, isl], outa2[:, :],
                             start=True, stop=True)
            nc.vector.tensor_copy(agI2[:, it, OUT:OUT + 1], f2p[:, :])
            transpose_to(agI2[:, it, 0:OUT], wh2b[:, isl], OUT)
        agIn2 = dram.tile([NS, OUT + 1], BF16)
        nc.gpsimd.dma_start(
            agIn2[:, :].rearrange("(it p) c -> p it c", p=128), agI2[:, :, :])
        agOut2 = dram.tile([N, OUT + 1], BF16, addr_space="Shared")
        nc.gpsimd.collective_compute(
            "AllGather", ALU.bypass, replica_groups=RG,
            ins=[agIn2.opt()], outs=[agOut2.opt()])
        agO2 = gnn.tile([128, JT, OUT + 1], BF16)
        nc.gpsimd.dma_start(
            agO2[:, :, :], agOut2[:, :].rearrange("(jt p) c -> p jt c", p=128))
        f2of = gnn.tile([128, JT, 1], F32)
        nc.vector.tensor_copy(f2of[:, :, :], agO2[:, :, OUT:OUT + 1])

        uz2 = pstile([64, 2 * NS], F32, "att")
        for jb in range(2):
            jts = range(jb * 8, jb * 8 + 8)
            zs, ls, ems = {}, {}, {}
            for jt in jts:
                zs[jt] = sbG.tile([128, NS], F32, tag="zsb", bufs=5,
                                  name=f"oz_{jt}")
                nc.scalar.activation(zs[jt][:, :], f1ob[:, :], AF.Identity,
                                     bias=f2of[:, jt, :], scale=1.0)
            for jt in jts:
                ls[jt] = sbG.tile([128, NS], F32, tag="lsb", bufs=5,
                                  name=f"ol_{jt}")
                nc.vector.scalar_tensor_tensor(
                    ls[jt][:, :], zs[jt][:, :], 0.2, zs[jt][:, :],
                    op0=ALU.mult, op1=ALU.max)
            for jt in jts:
                zs[jt] = sbG.tile([128, NS], BF16, tag="esb", bufs=4,
                                  name=f"oe_{jt}")
                nc.scalar.activation(zs[jt][:, :], ls[jt][:, :], AF.Exp)
            for jt in jts:
                ems[jt] = sbG.tile([128, NS], BF16, tag="em", bufs=4,
                                   name=f"om_{jt}")
                nc.vector.tensor_mul(ems[jt][:, :], zs[jt][:, :],
                                     adjs[:, jt, :])
            for jt in jts:
                nc.tensor.matmul(uz2[:, 0:NS], agO2[:, jt, 0:OUT],
                                 ems[jt][:, :],
                                 start=(jt == 0), stop=(jt == JT - 1))
                nc.tensor.matmul(uz2[0:1, NS:2 * NS], ones_col[:, :],
                                 ems[jt][:, :],
                                 start=(jt == 0), stop=(jt == JT - 1))
        z2iv = sbG.tile([1, NS], F32, tag="et", bufs=5, name="z2iv")
        nc.vector.reciprocal(z2iv[:, :], uz2[0:1, NS:2 * NS])
        z2bc = pstile([64, NS], F32, "tp")
        nc.tensor.matmul(z2bc[:, :], ones_row_f[:, 0:64], z2iv[:, :],
                         start=True, stop=True)
        z2bs = sbG.tile([64, NS], F32, tag="et", bufs=5, name="z2bs")
        nc.vector.tensor_copy(z2bs[:, :], z2bc[:, :])
        ho2 = sbG.tile([64, NS], F32, tag="et", bufs=5, name="ho2")
        nc.vector.tensor_mul(ho2[:, :], uz2[:, 0:NS], z2bs[:, :])
        mn2 = sbG.tile([64, NS], F32, tag="et", bufs=5, name="mn2")
        nc.vector.tensor_scalar_min(mn2[:, :], ho2[:, :], 0.0)
        ex2 = sbG.tile([64, NS], F32, tag="et", bufs=5, name="ex2")
        nc.scalar.activation(ex2[:, :], mn2[:, :], AF.Exp)
        ep2 = sbG.tile([64, NS], F32, tag="et", bufs=5, name="ep2")
        nc.vector.scalar_tensor_tensor(ep2[:, :], ho2[:, :], 0.0, ex2[:, :],
                                       op0=ALU.max, op1=ALU.add)
        gatx = gnn.tile([64, NS], F32)
        # lrelu(BN*(ep2-1), 0.01) = lrelu(BN*ep2 - BN, 0.01)
        nc.scalar.activation(gatx[:, :], ep2[:, :], AF.Lrelu,
                             bias=negbn[:, :], scale=BN, alpha=0.01)

        mark("fuse")
        # ---- fusion ----
        fu = sbG
        S = fu.tile([64, NS], F32, tag="fu", bufs=4, name="Sf")
        nc.vector.tensor_add(S[:, :], gatx[:, :], gcnx[:, :])
        base = fu.tile([64, NS], F32, tag="fu", bufs=4, name="basef")
        nc.vector.tensor_scalar(base[:, :], S[:, :], fuse[:, 5:6],
                                fuse[:, 2:3], op0=ALU.mult, op1=ALU.add)
        g1 = fu.tile([64, NS], F32, tag="fu", bufs=4, name="g1f")
        nc.vector.scalar_tensor_tensor(g1[:, :], gcnx[:, :], fuse[:, 0:1],
                                       base[:, :], op0=ALU.mult, op1=ALU.add)
        g2 = fu.tile([64, NS], F32, tag="fu", bufs=4, name="g2f")
        nc.vector.scalar_tensor_tensor(g2[:, :], gatx[:, :], fuse[:, 1:2],
                                       g1[:, :], op0=ALU.mult, op1=ALU.add)
        mnf = fu.tile([64, NS], F32, tag="fu", bufs=4, name="mnff")
        nc.vector.tensor_tensor(mnf[:, :], gcnx[:, :], gatx[:, :], op=ALU.min)
        mxf = fu.tile([64, NS], F32, tag="fu", bufs=4, name="mxff")
        nc.vector.tensor_tensor(mxf[:, :], gcnx[:, :], gatx[:, :], op=ALU.max)
        g3 = fu.tile([64, NS], F32, tag="fu", bufs=4, name="g3f")
        nc.vector.scalar_tensor_tensor(g3[:, :], mnf[:, :], fuse[:, 3:4],
                                       g2[:, :], op0=ALU.mult, op1=ALU.add)
        yf = fu.tile([64, NS], F32, tag="fu", bufs=4, name="yff")
        nc.vector.scalar_tensor_tensor(yf[:, :], mxf[:, :], fuse[:, 4:5],
                                       g3[:, :], op0=ALU.mult, op1=ALU.add)
        ybn = fu.tile([64, NS], F32, tag="fu", bufs=4, name="ybnf")
        nc.scalar.mul(ybn[:, :], yf[:, :], BN)
        yT = fu.tile([64, NS], F32, tag="fu", bufs=4, name="yTf")
        nc.vector.scalar_tensor_tensor(yT[:, :], ybn[:, :], 0.2, ybn[:, :],
                                       op0=ALU.mult, op1=ALU.max)
        yTb = fu.tile([64, NS], BF16, tag="fub", name="yTbf")
        nc.vector.tensor_copy(yTb[:, :], yT[:, :])
        # transpose to natural, AllGather y
        agYi = gnn.tile([128, 2, OUT], BF16)
        for it in range(2):
            transpose_to(agYi[:, it, :], yTb[:, it * 128:(it + 1) * 128],
                         OUT)
        agYIn = dram.tile([NS, OUT], BF16)
        nc.gpsimd.dma_start(
            agYIn[:, :].rearrange("(it p) c -> p it c", p=128), agYi[:, :, :])
        yD = dram.tile([N, OUT], BF16, addr_space="Shared")
        nc.gpsimd.collective_compute(
            "AllGather", ALU.bypass, replica_groups=RG,
            ins=[agYIn.opt()], outs=[yD.opt()])
        nc.gpsimd.dma_start(
            ysb[:, :, :], yD[:, :].rearrange("(jt p) f -> p jt f", p=128))
        gnn_blk.close()

        mark("C")
        # ---- phase C: outT[f, p] = sum_n y[n, f] Q.T[n, p] ----
        with tc.tile_pool(name="psC", bufs=1, space="PSUM") as psC, \
             tc.tile_pool(name="sbO", bufs=1) as sbO:
            osb = sbO.tile([64, 16, 512], F32)
            for qr in range(4):
                psT = psC.tile([64, 4, 512], F32, tag="psT", bufs=2,
                               name=f"psT{qr}")
                for jt in range(JT):
                    for pb in range(4):
                        nc.tensor.matmul(
                            psT[:, pb, :], ysb[:, jt, :],
                            qtts[jt][qr][:, pb * 512:(pb + 1) * 512],
                            start=(jt == 0), stop=(jt == JT - 1))
                nc.vector.tensor_copy(osb[:, qr * 4:(qr + 1) * 4, :],
                                      psT[:, :, :])
            nc.sync.dma_start(
                out_d[:, :].rearrange("f (b c) -> f b c", c=512),
                osb[:, :, :])
        mark(None)

    nc.compile()
    return nc


def _prep_inputs(x, adj, Q, g1a_W, g1a_b, g1b_W, g1b_b, g2a_W, g2a_b,
                 g2b_W, g2b_b, gat_W, gat_a, out_W, out_a, Wf, bf,
                 conv_w, conv_b):
    bft = ml_dtypes.bfloat16
    f32 = np.float32
    xf = np.asarray(x, f32).reshape(HW, C)
    Qb = np.asarray(Q, f32).astype(bft)
    adj = np.asarray(adj, f32)

    w1a = (BN ** 3 * np.asarray(g1a_W, f32)).astype(bft)
    w1b = (BN * np.asarray(g1b_W, f32)).astype(bft)
    w2a = (BN * np.asarray(g2a_W, f32)).astype(bft)
    w2b = (BN * np.asarray(g2b_W, f32)).astype(bft)
    gatw = np.concatenate([BN * np.asarray(gat_W[k], f32)
                           for k in range(HEADS)], axis=1).astype(bft)
    gat_Wf = np.asarray(gat_W, f32)
    gat_af = np.asarray(gat_a, f32)
    wa12 = np.stack(
        [BN * gat_Wf[k] @ gat_af[k, :NHID] for k in range(HEADS)] +
        [BN * gat_Wf[k] @ gat_af[k, NHID:] for k in range(HEADS)],
        axis=1).astype(bft)
    outw = np.asarray(out_W, f32).astype(bft)
    outa1 = np.tile(np.asarray(out_a[:OUT], f32)[:, None], (1, 128)).astype(bft)
    outa2 = np.asarray(out_a[OUT:], f32)[:, None].astype(bft)
    b1a = np.asarray(g1a_b, f32)[:, None]
    b1b = (BN * np.asarray(g1b_b, f32))[:, None]
    b2a = np.asarray(g2a_b, f32)[:, None]
    b2b = (BN * np.asarray(g2b_b, f32))[:, None]
    cw = np.asarray(conv_w, f32)
    cb = float(np.asarray(conv_b, f32)[0])
    Wf = np.asarray(Wf, f32)
    bfv = np.asarray(bf, f32)
    fuse = np.stack([
        cw[2] * Wf[0], cw[2] * Wf[1],
        cw[2] * (bfv[0] + bfv[1]) + cb,
        np.full(OUT, cw[0], f32), np.full(OUT, cw[1], f32),
        np.full(OUT, cw.sum(), f32),
    ], axis=1).astype(f32)

    adjf = np.ascontiguousarray(adj.T).astype(bft)

    shared = dict(w1a=w1a, w1b=w1b, w2a=w2a, w2b=w2b, gatw=gatw,
                  wa12=wa12, outw=outw, outa1=outa1,
                  outa2=outa2, b1a=b1a, b1b=b1b, b2a=b2a, b2b=b2b,
                  fuse=fuse, adjf=adjf)

    onespad = np.zeros((P, CP - C), f32)
    onespad[:, -1] = 1.0
    in_maps = []
    for c in range(NCORES):
        m = dict(shared)
        psl = slice(c * P, (c + 1) * P)
        m["xq"] = np.ascontiguousarray(
            np.concatenate([xf[psl], onespad], axis=1)).astype(bft)
        m["q"] = np.ascontiguousarray(Qb[psl])
        m["qt"] = np.ascontiguousarray(Qb[psl].T)
        m["adjs"] = np.ascontiguousarray(
            adj[c * NS:(c + 1) * NS, :].T.astype(bft))
        sel = np.zeros((N, NS), f32)
        sel[c * NS + np.arange(NS), np.arange(NS)] = 1.0
        m["sel"] = sel.astype(bft)
        in_maps.append(m)
    return in_maps


def _get_nc():
    if "nc" not in _CACHE:
        _CACHE["nc"] = _build()
    return _CACHE["nc"]


def run_traced(trace=False, **inputs):
    nc = _get_nc()
    in_maps = _prep_inputs(**inputs)
    res = run_bass_kernel_spmd(nc, in_maps, core_ids=list(range(NCORES)),
                               trace=trace)
    out = np.concatenate([res.results[c]["out"].T for c in range(NCORES)],
                         axis=0)
    return out, res


def kernel(**inputs):
    out, _ = run_traced(trace=False, **inputs)
    return out
